# revision 11
# baseline (speedup 1.0000x reference)
"""Trainium2 Bass kernel for CAConv2 (coordinate-attention + 3x3 conv block).

Shapes (hardcoded): x (8, 128, 128, 128) f32; data-parallel over batch,
one image per NeuronCore (8 cores).
"""

import numpy as np
import ml_dtypes

import concourse.bacc as bacc
import concourse.tile as tile
from concourse import mybir
from concourse.bass import ds
from concourse.bass_utils import run_bass_kernel_spmd
from concourse.tile_rust import add_dep_helper

BF16 = mybir.dt.bfloat16
F32 = mybir.dt.float32
C, H, W, MIP = 128, 128, 128, 8
WP = W + 4  # padded width: cols [2, 130) hold data, 0/1 and 130/131 are zero
HP = H + 2  # padded height: rows [1, 129) hold data
EPS = 1e-5
AF = mybir.ActivationFunctionType
ALU = mybir.AluOpType

_CACHE = {}


def build_nc():
    nc = bacc.Bacc()
    xp = nc.declare_dram_parameter("x", [C, H * W], BF16, isOutput=False)
    w1t = nc.declare_dram_parameter("w1t", [C, MIP], BF16, isOutput=False)
    wht = nc.declare_dram_parameter("wht", [MIP, C], BF16, isOutput=False)
    wwt = nc.declare_dram_parameter("wwt", [MIP, C], BF16, isOutput=False)
    # wct[i, k, o] = wc[o, i, k//3, k%3]
    wct = nc.declare_dram_parameter("wct", [C, 9 * C], BF16, isOutput=False)
    # p8 cols: 0: s1/6, 1: t1f/6, 2: s1, 3: t1f+3   (t1f = s1*b1 + be1 - m1*s1)
    p8 = nc.declare_dram_parameter("p8", [MIP, 4], F32, isOutput=False)
    # p128 cols: 0: bh, 1: bw, 2: s2, 3: b2 (= bc*s2 + be2 - m2*s2)
    p128 = nc.declare_dram_parameter("p128", [C, 4], F32, isOutput=False)
    outp = nc.declare_dram_parameter("out", [C, H, W], F32, isOutput=True)

    c1, c2, c3 = 7.0 / 128, 3.0 / 128, 1.0 / 128

    NCH = 8
    RPC = H // NCH  # rows per chunk = 16

    with tile.TileContext(nc) as tc:
        with (
            tc.tile_pool(name="sing", bufs=1) as sing,
            tc.tile_pool(name="pp", bufs=2) as pp,
            tc.tile_pool(name="small", bufs=1) as small,
        ):
            xs = sing.tile([C, H * W], BF16)
            ug = sing.tile([C, HP, WP], BF16)
            s32 = sing.tile([C, H, 4], F32)

            # w1t first (needed by the earliest matmuls), then the x chunks
            # with a depth-2 dependency chain so early chunks complete early.
            w1t_sb = sing.tile([C, MIP], BF16)
            nc.sync.dma_start(out=w1t_sb, in_=w1t[:, :])
            # x chunks: first two on the sync HWDGE ring (starts earliest),
            # the rest on the gpsimd SWDGE ring where descriptors drain in
            # issue order per engine slot -> staggered completion at full BW.
            for c in range(NCH):
                eng = nc.sync if c < 2 else nc.gpsimd
                eng.dma_start(
                    out=xs[:, ds(c * RPC * W, RPC * W)],
                    in_=xp[:, ds(c * RPC * W, RPC * W)],
                )

            # remaining weights / params
            wht_sb = sing.tile([MIP, C], BF16)
            nc.sync.dma_start(out=wht_sb, in_=wht[:, :])
            wwt_sb = sing.tile([MIP, C], BF16)
            nc.sync.dma_start(out=wwt_sb, in_=wwt[:, :])
            p8_sb = sing.tile([MIP, 4], F32)
            nc.sync.dma_start(out=p8_sb, in_=p8[:, :])
            p128_sb = sing.tile([C, 4], F32)
            nc.sync.dma_start(out=p128_sb, in_=p128[:, :])
            wct_sb = sing.tile([C, 9, C], BF16)
            nc.sync.dma_start(out=wct_sb, in_=wct.rearrange("i (k o) -> i k o", k=9))

            # ---- zero the conv padding border of ug (vector; gpsimd is busy
            # generating the x-chunk SWDGE descriptors)
            nc.vector.memset(ug[:, 0, :], 0.0)
            nc.vector.memset(ug[:, HP - 1, :], 0.0)
            nc.vector.memset(ug[:, 1 : HP - 1, 0:2], 0.0)
            nc.vector.memset(ug[:, 1 : HP - 1, WP - 2 : WP], 0.0)

            # ---- preload ACT function tables off the critical path ----
            dummy = small.tile([C, 2], F32)
            nc.vector.memset(dummy, 0.0)
            dump = small.tile([C, 2], F32)
            for fn in (AF.Sigmoid, AF.Silu):
                nc.scalar.activation(dump, dummy, fn, bias=0.0, scale=1.0)

            with tc.tile_pool(name="psA", bufs=1, space="PSUM") as psA:
                # four row ranges: [0,32) w=c1, [32,64) w=c2, [64,96)+[96,128) w=c3
                psxw = [
                    psA.tile([MIP, 4, W], F32, name=f"xw{r}", tag=f"xw{r}")
                    for r in range(4)
                ]
                ps_yh = psA.tile([MIP, H], F32, tag="yh")
                Ar = [
                    small.tile([MIP, W], F32, name=f"Ar{r}", tag=f"Ar{r}")
                    for r in range(4)
                ]

                def fold_range(r):
                    # sum the 4 j-classes of psxw[r] -> Ar[r]
                    cp = pp.tile([MIP, 4, W], F32, tag="xwcp")
                    nc.vector.tensor_copy(cp, psxw[r])
                    f1 = pp.tile([MIP, 2, W], F32, tag="xwf1")
                    nc.vector.tensor_add(f1, cp[:, 0:2, :], cp[:, 2:4, :])
                    nc.vector.tensor_add(Ar[r], f1[:, 0, :], f1[:, 1, :])

                for c in range(NCH):
                    # x_w partial sums: project through w1, accumulate rows.
                    for b in range(RPC // 4):
                        gb = c * (RPC // 4) + b
                        r = gb // 8
                        nc.tensor.matmul(
                            psxw[r],
                            w1t_sb,
                            xs[:, ds(gb * 4 * W, 4 * W)],
                            start=(gb % 8 == 0),
                            stop=(gb % 8 == 7),
                        )
                    if c % 2 == 1:
                        fold_range(c // 2)
                    # x_h 32-col segment sums via add tree (bf16, 2x mode)
                    xc = xs[:, ds(c * RPC * W, RPC * W)].rearrange(
                        "p (y q s) -> p y q s", q=4, s=32
                    )
                    t1 = pp.tile([C, RPC, 4, 16], BF16, tag="t1")
                    nc.vector.tensor_add(t1, xc[:, :, :, 0:16], xc[:, :, :, 16:32])
                    t2 = pp.tile([C, RPC, 4, 8], BF16, tag="t2")
                    nc.vector.tensor_add(t2, t1[:, :, :, 0:8], t1[:, :, :, 8:16])
                    t3 = pp.tile([C, RPC, 4, 4], BF16, tag="t3")
                    nc.vector.tensor_add(t3, t2[:, :, :, 0:4], t2[:, :, :, 4:8])
                    t4 = pp.tile([C, RPC, 4, 2], BF16, tag="t4")
                    nc.vector.tensor_add(t4, t3[:, :, :, 0:2], t3[:, :, :, 2:4])
                    sl = s32[:, ds(c * RPC, RPC), :]
                    nc.vector.tensor_add(sl, t4[:, :, :, 0], t4[:, :, :, 1])

                    # per-chunk x_h combine + w1 projection of this row slice
                    tmpA = pp.tile([C, RPC], F32, tag="tmpA")
                    nc.vector.tensor_add(tmpA, sl[:, :, 2], sl[:, :, 3])
                    m0 = pp.tile([C, RPC], F32, tag="m0")
                    nc.vector.tensor_scalar_mul(m0, sl[:, :, 0], c1)
                    m1 = pp.tile([C, RPC], F32, tag="m1")
                    nc.vector.scalar_tensor_tensor(
                        out=m1, in0=sl[:, :, 1], scalar=c2, in1=m0,
                        op0=ALU.mult, op1=ALU.add,
                    )
                    xhp = pp.tile([C, RPC], BF16, tag="xhp")
                    nc.vector.scalar_tensor_tensor(
                        out=xhp, in0=tmpA, scalar=c3, in1=m1,
                        op0=ALU.mult, op1=ALU.add,
                    )
                    nc.tensor.matmul(
                        ps_yh[:, ds(c * RPC, RPC)], w1t_sb, xhp,
                        start=True, stop=True,
                    )

                # ---- x_w: combine ranges (P01 precombined; A2+A3 late) ----
                xw01 = small.tile([MIP, W], F32)
                nc.vector.tensor_scalar_mul(xw01, Ar[0], c1)
                nc.vector.scalar_tensor_tensor(
                    out=xw01, in0=Ar[1], scalar=c2, in1=xw01, op0=ALU.mult, op1=ALU.add
                )
                a23 = small.tile([MIP, W], F32)
                nc.vector.tensor_add(a23, Ar[2], Ar[3])
                xwp = small.tile([MIP, W], F32)
                nc.vector.scalar_tensor_tensor(
                    out=xwp, in0=a23, scalar=c3, in1=xw01, op0=ALU.mult, op1=ALU.add
                )

                # ---- BN1 + h_swish on both (8, l) paths (all DVE) ----
                def bn_hswish(src, dst_tag):
                    z6 = small.tile([MIP, H], F32, tag=dst_tag + "_z6")
                    nc.vector.tensor_scalar(
                        out=z6, in0=src, scalar1=p8_sb[:, 0:1],
                        scalar2=p8_sb[:, 1:2], op0=ALU.mult, op1=ALU.add,
                    )
                    r = small.tile([MIP, H], F32, tag=dst_tag + "_r")
                    nc.vector.tensor_scalar(
                        out=r, in0=z6, scalar1=6.0, scalar2=3.0,
                        op0=ALU.mult, op1=ALU.add,
                    )
                    rc = small.tile([MIP, H], F32, tag=dst_tag + "_rc")
                    nc.vector.tensor_scalar(
                        out=rc, in0=r, scalar1=0.0, scalar2=6.0,
                        op0=ALU.max, op1=ALU.min,
                    )
                    dst = small.tile([MIP, H], BF16, tag=dst_tag)
                    nc.vector.tensor_mul(dst, z6, rc)
                    return dst

                xh_s = bn_hswish(ps_yh, "xh_s")
                xw_s = bn_hswish(xwp, "xw_s")

                # ---- attention gates ----
                ps_ah = psA.tile([C, H], F32, tag="ah")
                nc.tensor.matmul(ps_ah, wht_sb, xh_s, start=True, stop=True)
                ah_sb = small.tile([C, H], BF16)
                nc.scalar.activation(
                    ah_sb, ps_ah, AF.Sigmoid, bias=p128_sb[:, 0:1], scale=1.0
                )
                ps_aw = psA.tile([C, W], F32, tag="aw")
                nc.tensor.matmul(ps_aw, wwt_sb, xw_s, start=True, stop=True)
                aw_sb = small.tile([C, W], BF16)
                nc.scalar.activation(
                    aw_sb, ps_aw, AF.Sigmoid, bias=p128_sb[:, 1:2], scale=1.0
                )

                # ---- gating: ug = x * a_h[c,y] * a_w[c,x], one fused op per row
                for y in range(H):
                    nc.vector.scalar_tensor_tensor(
                        out=ug[:, 1 + y, 2 : 2 + W],
                        in0=xs[:, ds(y * W, W)],
                        scalar=ah_sb[:, y : y + 1],
                        in1=aw_sb,
                        op0=ALU.mult,
                        op1=ALU.mult,
                    )

            # ---- 3x3 conv + BN2 + SiLU ----
            with (
                tc.tile_pool(name="psB", bufs=3, space="PSUM") as psB,
                tc.tile_pool(name="obp", bufs=3) as obp,
            ):
                for rb in range(H // 4):
                    pso = psB.tile([C, 4, W], F32, tag="pso")
                    for k in range(9):
                        dy, dx = k // 3, k % 3
                        nc.tensor.matmul(
                            pso,
                            wct_sb[:, k, :],
                            ug[:, 4 * rb + dy : 4 * rb + dy + 4, 1 + dx : 1 + dx + W],
                            start=(k == 0),
                            stop=(k == 8),
                        )
                    ob = obp.tile([C, 4, W], F32, tag="ob")
                    nc.scalar.activation(
                        ob, pso, AF.Silu, bias=p128_sb[:, 3:4], scale=p128_sb[:, 2:3]
                    )
                    nc.sync.dma_start(out=outp[:, 4 * rb : 4 * rb + 4, :], in_=ob)

    nc.compile()
    return nc


def prep_inputs(x, w1, b1, g1, be1, m1, v1, wh, bh, ww, bw, wc, bc, g2, be2, m2, v2):
    """Host-side prep: per-core input maps (weights replicated)."""
    bf = ml_dtypes.bfloat16
    N = x.shape[0]
    s1 = (g1 / np.sqrt(v1 + EPS)).astype(np.float64)
    t1f = s1 * b1 + be1 - m1 * s1
    p8 = np.stack([s1 / 6.0, t1f / 6.0, s1, t1f + 3.0], axis=1).astype(np.float32)
    s2 = (g2 / np.sqrt(v2 + EPS)).astype(np.float64)
    b2 = bc * s2 + be2 - m2 * s2
    p128 = np.stack([bh, bw, s2, b2], axis=1).astype(np.float32)
    shared = {
        "w1t": np.ascontiguousarray(w1.T).astype(bf),            # (C, MIP)
        "wht": np.ascontiguousarray(wh.T).astype(bf),            # (MIP, C)
        "wwt": np.ascontiguousarray(ww.T).astype(bf),            # (MIP, C)
        "wct": np.ascontiguousarray(
            np.transpose(wc, (1, 2, 3, 0)).reshape(C, 9 * C)
        ).astype(bf),                                            # [i, (ky kx), o]
        "p8": p8,
        "p128": p128,
    }
    in_maps = []
    for n in range(N):
        m = dict(shared)
        m["x"] = np.ascontiguousarray(x[n].reshape(C, H * W)).astype(bf)
        in_maps.append(m)
    return in_maps


def run(inputs, trace=False):
    if "nc" not in _CACHE:
        _CACHE["nc"] = build_nc()
    nc = _CACHE["nc"]
    in_maps = prep_inputs(**inputs)
    res = run_bass_kernel_spmd(nc, in_maps, core_ids=list(range(8)), trace=trace)
    out = np.stack([np.asarray(res.results[i]["out"]) for i in range(8)], axis=0)
    return out.astype(np.float32), res


def kernel(**inputs) -> np.ndarray:
    out, _ = run(inputs, trace=False)
    return out
